# revision 1
# baseline (speedup 1.0000x reference)
"""GAT-style 2-conv GNN forward on 8 Trainium2 NeuronCores.

Strategy (graph/data parallel per the sharding hint):
  - Nodes partitioned across 8 cores by destination range. Each core computes
    dense per-node features for its slice (x0 = relu(x@W+b); packed row
    [h1|s1|h2|s2|d1|d2] in a 128-col fp16 row = 256B), then an AllGather
    replicates the full node-feature table.
  - Per-edge gathers use the InstDMAGatherAnt primitive (int16 indices), so
    the table is addressed through 4 windows of 2 cores (25088 rows < 32768).
    For window q, each core reorders its nodes by window-q in-degree (shared
    per-tile degree profile across cores -> one SPMD program), gathers source
    rows for its incoming edges in a [128, slots] grid (node-per-partition,
    slots along free dim), computes ex = exp(leakyrelu(s_src + d_dst)) and a
    strided segmented reduce -> per-window partial numerators/denominators,
    written to local HBM in window order.
  - Partials are re-gathered (local dma_gather) into common node order and
    summed; softmax max-subtraction is skipped (logits are O(5), exp cannot
    overflow; algebraically identical).
  - Final: out = log_softmax([x1 | x2 | x3] + x) per node tile.

Host work is index/layout preprocessing only (sort, pad, permute, weight
concatenation); all network floating-point math runs on device.
"""

import sys

sys.path.insert(0, "/opt/trn_rl_repo")

import contextlib

import numpy as np

import concourse.bacc as bacc
import concourse.bass as bass
import concourse.bass_utils as bass_utils
import concourse.mybir as mybir
import concourse.tile as tile
from concourse import library_config
from concourse.masks import make_identity

FP32 = mybir.dt.float32
FP16 = mybir.dt.float16
INT16 = mybir.dt.int16

N_CORES = 8
N_WIN = 4
P = 128
ELEM = 128  # fp16 cols per table row = 256B

# table row columns
S1C, S2C, D1C, D2C = 32, 65, 66, 67
H1B, H2B = 0, 33
# partial row columns
PN1, PN2, PD1, PD2 = 0, 32, 64, 65
NEG_SLOPE = 0.2
DUMMY_S = -30000.0

S_CHUNK = 96  # grid columns per chunk (96*128 = 12288 idxs <= HW gather limit)


def _wrap16(stream):
    """[n] -> [128, n//16] int16 in the 16-partition wrapped+replicated layout."""
    n = stream.shape[0]
    assert n % 16 == 0
    w = stream.reshape(n // 16, 16).T.astype(np.int16)  # [16, n//16]
    return np.tile(w, (8, 1))


def _build_layout(edge_index, n_nodes):
    src = np.asarray(edge_index[0], dtype=np.int64)
    dst = np.asarray(edge_index[1], dtype=np.int64)
    E = src.shape[0]

    npc_raw = -(-n_nodes // N_CORES)
    tiles = -(-npc_raw // P)
    npc = tiles * P
    if npc == npc_raw:  # ensure pad rows exist (used as harmless dummies)
        tiles += 1
        npc += P
    n_pad = npc * N_CORES
    wsize = 2 * npc
    assert wsize <= 32768

    # map old node id -> (core, local) by original contiguous ranges
    core_of = np.minimum(dst // npc_raw, N_CORES - 1)
    # new global id (common order): core*npc + (old - core*npc_raw)
    old2new = np.empty(n_nodes, dtype=np.int64)
    for c in range(N_CORES):
        lo = c * npc_raw
        hi = min(lo + npc_raw, n_nodes)
        old2new[lo:hi] = c * npc + np.arange(hi - lo)

    new_src = old2new[src]
    new_dst = old2new[dst]
    dst_core = new_dst // npc
    dst_local = new_dst % npc
    src_win = new_src // wsize

    # per (core, window) in-degree
    qdeg = np.zeros((N_CORES, N_WIN, npc), dtype=np.int64)
    np.add.at(qdeg, (dst_core, src_win, dst_local), 1)

    # per (core, window): node order by q-degree desc
    node_at = np.empty((N_CORES, N_WIN, npc), dtype=np.int64)
    pos_of = np.empty((N_CORES, N_WIN, npc), dtype=np.int64)
    for c in range(N_CORES):
        for q in range(N_WIN):
            o = np.argsort(-qdeg[c, q], kind="stable")
            node_at[c, q] = o
            pos_of[c, q, o] = np.arange(npc)

    # shared tile degree profile per window
    D_q = np.zeros((N_WIN, tiles), dtype=np.int64)
    for q in range(N_WIN):
        sorted_deg = np.take_along_axis(qdeg[:, q, :], node_at[:, q, :], axis=1)
        D_q[q] = sorted_deg[:, ::P].max(axis=0)

    fb_q = np.zeros((N_WIN, tiles + 1), dtype=np.int64)
    for q in range(N_WIN):
        fb_q[q, 1:] = np.cumsum(D_q[q])
    slots_q = fb_q[:, -1].copy()  # columns per window grid

    # edge -> grid cell
    pos = pos_of[dst_core, src_win, dst_local]  # [E]
    t = pos // P
    p = pos % P
    # rank j within (core, window, dst)
    key = ((dst_core * N_WIN + src_win) * npc + dst_local)
    order = np.argsort(key, kind="stable")
    sk = key[order]
    first = np.flatnonzero(np.r_[True, sk[1:] != sk[:-1]])
    group_start = np.repeat(first, np.diff(np.r_[first, E]))
    j = np.empty(E, dtype=np.int64)
    j[order] = np.arange(E) - group_start
    assert (j < D_q[src_win, t]).all()
    col = fb_q[src_win, t] + j

    # build per-core gidx streams (concatenated over windows)
    pad_rel = npc_raw  # first pad row of the window's first core, window-relative
    gidx = np.empty((N_CORES, P, int(slots_q.sum()) * 8), dtype=np.int16)
    wbase = np.concatenate([[0], np.cumsum(slots_q)])
    for c in range(N_CORES):
        for q in range(N_WIN):
            sq = int(slots_q[q])
            stream = np.full(sq * P, pad_rel, dtype=np.int64)
            m = (dst_core == c) & (src_win == q)
            stream[col[m] * P + p[m]] = new_src[m] - q * wsize
            assert stream.max() < wsize and stream.min() >= 0
            gidx[c, :, int(wbase[q]) * 8 : int(wbase[q] + sq) * 8] = _wrap16(
                stream
            )

    # d-gather / merge-gather index streams
    dgidx = np.empty((N_CORES, P, N_WIN * npc // 16), dtype=np.int16)
    mgidx = np.empty((N_CORES, P, N_WIN * npc // 16), dtype=np.int16)
    for c in range(N_CORES):
        for q in range(N_WIN):
            sl = slice(q * npc // 16, (q + 1) * npc // 16)
            dgidx[c, :, sl] = _wrap16(node_at[c, q])
            mgidx[c, :, sl] = _wrap16(pos_of[c, q])

    # chunk structure per window: runs of equal D (D>0), split/packed <= S_CHUNK
    win_chunks = []
    for q in range(N_WIN):
        runs = []
        t0 = 0
        for tt in range(1, tiles + 1):
            if tt == tiles or D_q[q, tt] != D_q[q, t0]:
                if D_q[q, t0] > 0:
                    runs.append((t0, tt - t0, int(D_q[q, t0])))
                t0 = tt
        pieces = []
        for (rt0, g, d) in runs:
            max_g = max(1, S_CHUNK // d)
            s = 0
            while s < g:
                gg = min(max_g, g - s)
                pieces.append((rt0 + s, gg, d))
                s += gg
        chunks = []
        cur, cur_cols = [], 0
        for pc in pieces:
            need = pc[1] * pc[2]
            assert need <= S_CHUNK
            if cur_cols + need > S_CHUNK:
                chunks.append(cur)
                cur, cur_cols = [], 0
            cur.append(pc)
            cur_cols += need
        if cur:
            chunks.append(cur)
        win_chunks.append(chunks)

    # zero-tile tail start per window (tiles with D==0 need zeroed partials)
    zstart = []
    for q in range(N_WIN):
        nz = np.flatnonzero(D_q[q] > 0)
        zstart.append(int(nz[-1]) + 1 if nz.size else 0)

    return dict(
        npc_raw=npc_raw, npc=npc, tiles=tiles, n_pad=n_pad, wsize=wsize,
        old2new=old2new, D_q=D_q, fb_q=fb_q, slots_q=slots_q,
        gidx=gidx, dgidx=dgidx, mgidx=mgidx, win_chunks=win_chunks,
        zstart=zstart,
    )


def _build_program(lay, f_in, hidden, ncls):
    tiles = lay["tiles"]
    npc = lay["npc"]
    npc_raw = lay["npc_raw"]
    n_pad = lay["n_pad"]
    wsize = lay["wsize"]
    slots_q = lay["slots_q"]
    fb_q = lay["fb_q"]
    win_chunks = lay["win_chunks"]
    F = 2 * ncls + 1
    assert F == f_in
    HC = 2 * ncls + 4  # used table columns

    nc = bacc.Bacc("TRN2", target_bir_lowering=False, debug=False,
                   enable_asserts=False, num_devices=N_CORES)

    xT_in = nc.dram_tensor("xT", [f_in, npc], FP32, kind="ExternalInput").ap()
    x_in = nc.dram_tensor("xrow", [npc, f_in], FP32, kind="ExternalInput").ap()
    wmlp_in = nc.dram_tensor("wmlp", [f_in, hidden], FP32, kind="ExternalInput").ap()
    bmlp_in = nc.dram_tensor("bmlp", [hidden, 1], FP32, kind="ExternalInput").ap()
    wcat_in = nc.dram_tensor("wcat", [hidden, HC], FP32, kind="ExternalInput").ap()
    bb_in = nc.dram_tensor("bb", [P, 2 * ncls], FP32, kind="ExternalInput").ap()
    padm_in = nc.dram_tensor("padm", [P, tiles], FP16, kind="ExternalInput").ap()
    gidx_in = nc.dram_tensor(
        "gidx", [P, int(slots_q.sum()) * 8], INT16, kind="ExternalInput"
    ).ap()
    dgidx_in = nc.dram_tensor(
        "dgidx", [P, N_WIN * npc // 16], INT16, kind="ExternalInput"
    ).ap()
    mgidx_in = nc.dram_tensor(
        "mgidx", [P, N_WIN * npc // 16], INT16, kind="ExternalInput"
    ).ap()
    out_t = nc.dram_tensor("out", [npc, F], FP32, kind="ExternalOutput").ap()

    with tile.TileContext(nc) as tc:
        with contextlib.ExitStack() as ctx:
            persist = ctx.enter_context(tc.tile_pool(name="persist", bufs=1))
            dram = ctx.enter_context(tc.tile_pool(name="dram", bufs=1, space="DRAM"))

            nc.gpsimd.load_library(library_config.mlp)

            x3buf = persist.tile([P, tiles], FP32)
            bb_sb = persist.tile([P, 2 * ncls], FP32)
            padm_sb = persist.tile([P, tiles], FP16)
            nc.sync.dma_start(out=bb_sb[:], in_=bb_in[:])
            nc.sync.dma_start(out=padm_sb[:], in_=padm_in[:])

            hloc_d = dram.tile([npc, ELEM], FP16)
            htab_d = dram.tile([n_pad, ELEM], FP16)
            part_d = [dram.tile([npc, ELEM], FP16, name=f"part{q}",
                                tag=f"part{q}")
                      for q in range(N_WIN)]

            # ---------------- Phase 1: dense local features ----------------
            with tc.tile_pool(name="ph1c", bufs=1) as cpool, \
                 tc.tile_pool(name="ph1", bufs=3) as ph1, \
                 tc.tile_pool(name="ph1x", bufs=2) as ph1x, \
                 tc.tile_pool(name="hl", bufs=1) as hlp, \
                 tc.tile_pool(name="ps1", bufs=2, space="PSUM") as ps1, \
                 tc.tile_pool(name="ps2", bufs=2, space="PSUM") as ps2, \
                 tc.tile_pool(name="ps3", bufs=2, space="PSUM") as ps3:
                wmlp_sb = cpool.tile([f_in, hidden], FP32)
                bmlp_sb = cpool.tile([hidden, 1], FP32)
                wcat_sb = cpool.tile([hidden, HC], FP32)
                ident = cpool.tile([P, P], FP32)
                nc.sync.dma_start(out=wmlp_sb[:], in_=wmlp_in[:])
                nc.sync.dma_start(out=bmlp_sb[:], in_=bmlp_in[:])
                nc.sync.dma_start(out=wcat_sb[:], in_=wcat_in[:])
                make_identity(nc, ident[:])

                hloc_sb = hlp.tile([P, tiles, ELEM], FP16)
                nc.vector.memset(hloc_sb[:], 0.0)

                XCH = 16
                for t in range(tiles):
                    if t % XCH == 0:
                        g = min(XCH, tiles - t)
                        xt_sb = ph1x.tile([f_in, XCH * P], FP32, tag="xt")
                        nc.sync.dma_start(
                            out=xt_sb[:, : g * P],
                            in_=xT_in[:, t * P : (t + g) * P],
                        )
                    xoff = (t % XCH) * P
                    psA = ps1.tile([P, P], FP32, space="PSUM")
                    nc.tensor.matmul(
                        out=psA[:], lhsT=wmlp_sb[:],
                        rhs=xt_sb[:, xoff : xoff + P],
                        start=True, stop=True,
                    )
                    x0t = ph1.tile([P, P], FP32, tag="x0t")
                    nc.scalar.activation(
                        out=x0t[:], in_=psA[:],
                        func=mybir.ActivationFunctionType.Relu,
                        bias=bmlp_sb[:, 0:1], scale=1.0,
                    )
                    psH = ps2.tile([P, HC], FP32, space="PSUM")
                    nc.tensor.matmul(
                        out=psH[:], lhsT=x0t[:], rhs=wcat_sb[:],
                        start=True, stop=True,
                    )
                    nc.vector.tensor_copy(out=hloc_sb[:, t, 0:HC], in_=psH[:])
                    psT = ps3.tile([P, P], FP32, space="PSUM")
                    nc.tensor.transpose(out=psT[:], in_=x0t[:], identity=ident[:])
                    nc.vector.tensor_reduce(
                        out=x3buf[:, t : t + 1], in_=psT[:],
                        axis=mybir.AxisListType.X, op=mybir.AluOpType.max,
                    )
                # force pad-row s columns to a huge negative (dummy target rows)
                for scol in (S1C, S2C):
                    nc.vector.tensor_tensor(
                        out=hloc_sb[:, :, scol : scol + 1],
                        in0=hloc_sb[:, :, scol : scol + 1],
                        in1=padm_sb[:].unsqueeze(2),
                        op=mybir.AluOpType.add,
                    )
                nc.sync.dma_start(
                    out=hloc_d[:].rearrange("(t p) c -> p t c", p=P),
                    in_=hloc_sb[:],
                )

            # ---------------- Phase 2: AllGather table ----------------
            nc.gpsimd.collective_compute(
                "AllGather",
                mybir.AluOpType.bypass,
                replica_groups=[list(range(N_CORES))],
                ins=[hloc_d[:].opt()],
                outs=[htab_d[:].opt()],
            )

            # ---------------- Phase 3: per-window gather + reduce ----------
            with tc.tile_pool(name="gi", bufs=1) as gip, \
                 tc.tile_pool(name="msg", bufs=2) as msgp, \
                 tc.tile_pool(name="sc", bufs=2) as scp, \
                 tc.tile_pool(name="exb", bufs=2) as exp_, \
                 tc.tile_pool(name="dg", bufs=2) as dgp, \
                 tc.tile_pool(name="dq", bufs=1) as dqp, \
                 tc.tile_pool(name="pt", bufs=2) as ptp:
                gidx_sb = gip.tile([P, int(slots_q.sum()) * 8], INT16)
                dgidx_sb = gip.tile([P, N_WIN * npc // 16], INT16)
                nc.sync.dma_start(out=gidx_sb[:], in_=gidx_in[:])
                nc.sync.dma_start(out=dgidx_sb[:], in_=dgidx_in[:])
                wbase = np.concatenate([[0], np.cumsum(slots_q)]).astype(int)

                for q in range(N_WIN):
                    # d values for this window's node order (local table rows)
                    dgt = dgp.tile([P, tiles, ELEM], FP16, tag="dgt")
                    t_half = (tiles + 1) // 2
                    for (tb, te) in ((0, t_half), (t_half, tiles)):
                        if te <= tb:
                            continue
                        nidx = (te - tb) * P
                        ib = q * npc // 16 + tb * P // 16
                        nc.gpsimd.dma_gather(
                            out_ap=dgt[:, tb:te, :],
                            in_ap=hloc_d[:],
                            idxs_ap=dgidx_sb[:, ib : ib + nidx // 16],
                            num_idxs=nidx,
                            num_idxs_reg=nidx,
                            elem_size=ELEM,
                            single_packet=False,
                        )
                    d1q = dqp.tile([P, tiles], FP32, tag=f"d1q{q}")
                    d2q = dqp.tile([P, tiles], FP32, tag=f"d2q{q}")
                    nc.vector.tensor_copy(out=d1q[:], in_=dgt[:, :, D1C])
                    nc.vector.tensor_copy(out=d2q[:], in_=dgt[:, :, D2C])

                    partial = ptp.tile([P, tiles, ELEM], FP16, tag="partial")
                    nc.vector.memset(partial[:], 0.0)

                    tab = htab_d[q * wsize : (q + 1) * wsize, :]
                    for chunk in win_chunks[q]:
                        ccols = sum(g * d for (_, g, d) in chunk)
                        cb = int(fb_q[q, chunk[0][0]])
                        msg = msgp.tile([P, S_CHUNK, ELEM], FP16, tag="msg")
                        ib = (int(wbase[q]) + cb) * 8
                        nc.gpsimd.dma_gather(
                            out_ap=msg[:, :ccols, :],
                            in_ap=tab,
                            idxs_ap=gidx_sb[:, ib : ib + ccols * 8],
                            num_idxs=ccols * P,
                            num_idxs_reg=ccols * P,
                            elem_size=ELEM,
                            single_packet=False,
                        )
                        for (rt0, g, d) in chunk:
                            s0 = int(fb_q[q, rt0]) - cb
                            mv = msg[:, s0 : s0 + g * d, :].rearrange(
                                "p (g e) c -> p g e c", e=d
                            )
                            for (hb, sc_, dq_, nb, db) in (
                                (H1B, S1C, d1q, PN1, PD1),
                                (H2B, S2C, d2q, PN2, PD2),
                            ):
                                exb = exp_.tile([P, S_CHUNK], FP32, tag="exb")
                                exv = exb[:, : g * d].rearrange(
                                    "p (g e) -> p g e", e=d
                                )
                                nc.vector.tensor_tensor(
                                    out=exv,
                                    in0=mv[:, :, :, sc_],
                                    in1=dq_[:, rt0 : rt0 + g]
                                    .unsqueeze(2)
                                    .broadcast_to([P, g, d]),
                                    op=mybir.AluOpType.add,
                                )
                                lrb = exp_.tile([P, S_CHUNK], FP32, tag="lrb")
                                lrv = lrb[:, : g * d].rearrange(
                                    "p (g e) -> p g e", e=d
                                )
                                nc.vector.tensor_scalar_mul(lrv, exv, NEG_SLOPE)
                                nc.vector.tensor_tensor(
                                    out=exv, in0=exv, in1=lrv,
                                    op=mybir.AluOpType.max,
                                )
                                nc.scalar.activation(
                                    out=exv, in_=exv,
                                    func=mybir.ActivationFunctionType.Exp,
                                )
                                sct = scp.tile(
                                    [P, S_CHUNK, ncls], FP32, tag="sc"
                                )
                                scv = sct[:, : g * d, :].rearrange(
                                    "p (g e) c -> p g e c", e=d
                                )
                                nc.vector.tensor_tensor(
                                    out=scv,
                                    in0=mv[:, :, :, hb : hb + ncls],
                                    in1=exv.unsqueeze(3).broadcast_to(
                                        [P, g, d, ncls]
                                    ),
                                    op=mybir.AluOpType.mult,
                                )
                                with nc.allow_low_precision("fp16 partials"):
                                    nc.vector.tensor_reduce(
                                        out=partial[:, rt0 : rt0 + g, nb : nb + ncls],
                                        in_=sct[:, : g * d, :].rearrange(
                                            "p (g e) c -> p g c e", e=d
                                        ),
                                        axis=mybir.AxisListType.X,
                                        op=mybir.AluOpType.add,
                                    )
                                    nc.vector.tensor_reduce(
                                        out=partial[:, rt0 : rt0 + g, db],
                                        in_=exv,
                                        axis=mybir.AxisListType.X,
                                        op=mybir.AluOpType.add,
                                    )
                    nc.sync.dma_start(
                        out=part_d[q][:].rearrange("(t p) c -> p t c", p=P),
                        in_=partial[:],
                    )

            # ---------------- Phase 3.9: merge partials --------------------
            with tc.tile_pool(name="mg", bufs=2) as mgp, \
                 tc.tile_pool(name="mgi", bufs=1) as mgip, \
                 tc.tile_pool(name="acc", bufs=1) as accp, \
                 tc.tile_pool(name="fin", bufs=1) as finp, \
                 tc.tile_pool(name="tmp", bufs=1) as tmpp:
                mgidx_sb = mgip.tile([P, N_WIN * npc // 16], INT16)
                nc.sync.dma_start(out=mgidx_sb[:], in_=mgidx_in[:])
                acc = accp.tile([P, tiles, 2 * ncls + 2], FP32)
                for q in range(N_WIN):
                    mg = mgp.tile([P, tiles, ELEM], FP16, tag="mg")
                    t_half = (tiles + 1) // 2
                    for (tb, te) in ((0, t_half), (t_half, tiles)):
                        if te <= tb:
                            continue
                        nidx = (te - tb) * P
                        ib = q * npc // 16 + tb * P // 16
                        nc.gpsimd.dma_gather(
                            out_ap=mg[:, tb:te, :],
                            in_ap=part_d[q][:],
                            idxs_ap=mgidx_sb[:, ib : ib + nidx // 16],
                            num_idxs=nidx,
                            num_idxs_reg=nidx,
                            elem_size=ELEM,
                            single_packet=False,
                        )
                    if q == 0:
                        nc.vector.tensor_copy(
                            out=acc[:], in_=mg[:, :, 0 : 2 * ncls + 2]
                        )
                    else:
                        nc.vector.tensor_tensor(
                            out=acc[:], in0=acc[:],
                            in1=mg[:, :, 0 : 2 * ncls + 2],
                            op=mybir.AluOpType.add,
                        )

                # ------------- Phase 4: normalize + residual + lsm ---------
                xin = finp.tile([P, tiles, F], FP32)
                nc.sync.dma_start(
                    out=xin[:], in_=x_in[:].rearrange("(t p) f -> p t f", p=P)
                )
                rden = tmpp.tile([P, tiles], FP32, tag="rden")
                for conv in range(2):
                    numv = acc[:, :, conv * ncls : (conv + 1) * ncls]
                    denv = acc[:, :, 2 * ncls + conv]
                    nc.vector.tensor_scalar_add(denv, denv, 1e-16)
                    nc.vector.reciprocal(out=rden[:], in_=denv)
                    nc.vector.tensor_tensor(
                        out=numv, in0=numv,
                        in1=rden[:].unsqueeze(2).broadcast_to([P, tiles, ncls]),
                        op=mybir.AluOpType.mult,
                    )
                    nc.vector.tensor_tensor(
                        out=numv, in0=numv,
                        in1=bb_sb[:, conv * ncls : (conv + 1) * ncls]
                        .unsqueeze(1)
                        .broadcast_to([P, tiles, ncls]),
                        op=mybir.AluOpType.add,
                    )
                    if conv == 0:
                        nc.vector.tensor_scalar_max(numv, numv, 0.0)
                    nc.vector.tensor_tensor(
                        out=xin[:, :, conv * ncls : (conv + 1) * ncls],
                        in0=xin[:, :, conv * ncls : (conv + 1) * ncls],
                        in1=numv,
                        op=mybir.AluOpType.add,
                    )
                nc.vector.tensor_tensor(
                    out=xin[:, :, 2 * ncls], in0=xin[:, :, 2 * ncls],
                    in1=x3buf[:], op=mybir.AluOpType.add,
                )
                mx = tmpp.tile([P, tiles], FP32, tag="mx")
                nc.vector.tensor_reduce(
                    out=mx[:], in_=xin[:], axis=mybir.AxisListType.X,
                    op=mybir.AluOpType.max,
                )
                nc.vector.tensor_tensor(
                    out=xin[:], in0=xin[:],
                    in1=mx[:].unsqueeze(2).broadcast_to([P, tiles, F]),
                    op=mybir.AluOpType.subtract,
                )
                et = tmpp.tile([P, tiles, F], FP32, tag="et")
                nc.scalar.activation(
                    out=et[:], in_=xin[:],
                    func=mybir.ActivationFunctionType.Exp,
                )
                sm = tmpp.tile([P, tiles], FP32, tag="sm")
                nc.vector.tensor_reduce(
                    out=sm[:], in_=et[:], axis=mybir.AxisListType.X,
                    op=mybir.AluOpType.add,
                )
                lg = tmpp.tile([P, tiles], FP32, tag="lg")
                nc.scalar.activation(
                    out=lg[:], in_=sm[:],
                    func=mybir.ActivationFunctionType.Ln,
                )
                nc.vector.tensor_tensor(
                    out=xin[:], in0=xin[:],
                    in1=lg[:].unsqueeze(2).broadcast_to([P, tiles, F]),
                    op=mybir.AluOpType.subtract,
                )
                nc.sync.dma_start(
                    out=out_t[:].rearrange("(t p) f -> p t f", p=P), in_=xin[:]
                )

    nc.compile()
    return nc


def _run(nc, lay, x, W_mlp, b_mlp, W1, a1_src, a1_dst, b1,
         W2, a2_src, a2_dst, b2, trace=False):
    n_nodes, f_in = x.shape
    hidden = W_mlp.shape[1]
    ncls = W1.shape[1]
    npc = lay["npc"]
    npc_raw = lay["npc_raw"]
    n_pad = lay["n_pad"]
    HC = 2 * ncls + 4

    xp = np.zeros((n_pad, f_in), dtype=np.float32)
    xp[lay["old2new"][: n_nodes]] = np.asarray(x, dtype=np.float32)

    wcat = np.concatenate(
        [W1, (W1 @ a1_src)[:, None], W2, (W2 @ a2_src)[:, None],
         (W1 @ a1_dst)[:, None], (W2 @ a2_dst)[:, None]],
        axis=1,
    ).astype(np.float32)
    assert wcat.shape == (hidden, HC)
    bb = np.broadcast_to(
        np.concatenate([b1, b2])[None, :], (P, 2 * ncls)
    ).astype(np.float32).copy()
    tiles = lay["tiles"]
    padm = np.zeros((npc,), dtype=np.float16)
    padm[npc_raw:] = DUMMY_S
    padm = np.ascontiguousarray(padm.reshape(tiles, P).T)  # [P, tiles]

    in_maps = []
    for c in range(N_CORES):
        xc = xp[c * npc : (c + 1) * npc]
        in_maps.append({
            "xT": np.ascontiguousarray(xc.T),
            "xrow": np.ascontiguousarray(xc),
            "wmlp": np.asarray(W_mlp, dtype=np.float32),
            "bmlp": np.asarray(b_mlp, dtype=np.float32)[:, None].copy(),
            "wcat": wcat,
            "bb": bb,
            "padm": padm,
            "gidx": np.ascontiguousarray(lay["gidx"][c]),
            "dgidx": np.ascontiguousarray(lay["dgidx"][c]),
            "mgidx": np.ascontiguousarray(lay["mgidx"][c]),
        })

    res = bass_utils.run_bass_kernel_spmd(
        nc, in_maps, core_ids=list(range(N_CORES)), trace=trace
    )
    outs = np.concatenate([r["out"] for r in res.results], axis=0)
    final = outs[lay["old2new"][: n_nodes]]
    return final, res


def kernel(x, edge_index, W_mlp, b_mlp, W1, a1_src, a1_dst, b1,
           W2, a2_src, a2_dst, b2, trace=False, _ret_res=False):
    x = np.asarray(x)
    lay = _build_layout(edge_index, x.shape[0])
    nc = _build_program(lay, x.shape[1], W_mlp.shape[1], W1.shape[1])
    out, res = _run(nc, lay, x, W_mlp, b_mlp, W1, a1_src, a1_dst, b1,
                    W2, a2_src, a2_dst, b2, trace=trace)
    if _ret_res:
        return out, res
    return out



# revision 5
# speedup vs baseline: 1.7599x; 1.7599x over previous
"""GAT-style 2-conv GNN forward on 8 Trainium2 NeuronCores (v2).

Strategy (graph/data parallel):
  - Nodes partitioned across 8 cores by destination range. Each core computes
    dense per-node features (packed 66-col fp16 row [h1|h2|s1|s2]); the table
    is replicated via two half AllGathers (first half overlaps phase-1's
    second half; second AllGather overlaps window-0 edge processing).
  - Table rows are grouped [all cores' first halves | all cores' second
    halves] so each 25088-row int16-addressable window is completed by one
    half-AllGather.
  - Per-edge gathers use InstDMAGatherAnt round-robined over 4 SWDGE queues
    (descriptor generation is the bottleneck at ~7.8ns/idx per queue; 4
    queues generate in parallel).
  - Per (core, window), destination nodes are degree-sorted into a dense
    [128 x slots] grid (shared tile-degree profile across cores -> one SPMD
    program). Slots are (e,g)-major within equal-degree runs so the segment
    reduces run in DVE 2x fp16 mode. Attention d-terms per window order are
    recomputed from host-permuted x via TensorEngine matmuls (no gather).
  - Window partials are merged back to local node order with one small
    gather per window half; softmax max-subtraction is skipped (logits are
    O(1); exp cannot overflow; algebraically identical).
  - Final: out = log_softmax([x1 | x2 | x3] + x) per node tile.

Host work is index/layout preprocessing only (sort, pad, permute, weight
concatenation); all network floating-point math runs on device.
"""

import sys

sys.path.insert(0, "/opt/trn_rl_repo")

import contextlib

import numpy as np

import concourse.bacc as bacc
import concourse.bass_utils as bass_utils
import concourse.mybir as mybir
import concourse.tile as tile
from concourse import library_config
from concourse.masks import make_identity

FP32 = mybir.dt.float32
FP16 = mybir.dt.float16
INT16 = mybir.dt.int16

N_CORES = 8
N_WIN = 4
P = 128
ELEM = 128  # fp16 cols per table row = 256B gather element
HC = 66  # used table cols [h1(32) | h2(32) | s1 | s2]
H1B, H2B, S12 = 0, 32, 64
NEG_SLOPE = 0.2
DUMMY_S = -30000.0
CH = 64  # target chunk columns (<= 96; small for queue granularity)


def _wrap16(stream):
    """[n] -> [128, n//16] int16 in the 16-partition wrapped+replicated layout."""
    n = stream.shape[0]
    assert n % 16 == 0
    w = stream.reshape(n // 16, 16).T.astype(np.int16)
    return np.tile(w, (8, 1))


def _build_layout(edge_index, n_nodes):
    src = np.asarray(edge_index[0], dtype=np.int64)
    dst = np.asarray(edge_index[1], dtype=np.int64)
    E = src.shape[0]

    npc_raw = -(-n_nodes // N_CORES)  # real nodes per core
    hs_raw = -(-npc_raw // 2)  # real nodes per half
    hs = -(-hs_raw // P) * P
    if hs == hs_raw:  # ensure pad rows exist in each half
        hs += P
    npc = 2 * hs
    tiles = npc // P
    htiles = hs // P
    n_pad = npc * N_CORES
    wsize = (N_CORES // 2) * hs
    assert wsize <= 32768
    pad_rel = hs_raw  # first pad row of a window's first core-slot

    # original node -> (core, device local)
    core_of = np.minimum(dst // npc_raw, N_CORES - 1)
    score = np.minimum(src // npc_raw, N_CORES - 1)

    def to_local(orig_local):
        return np.where(orig_local < hs_raw, orig_local, hs + orig_local - hs_raw)

    dst_local = to_local(dst - core_of * npc_raw)
    src_local = to_local(src - score * npc_raw)

    # source window: 2*(half) + (core//4); window-relative table row
    src_half = (src_local >= hs).astype(np.int64)
    src_win = 2 * src_half + (score // (N_CORES // 2))
    src_rel = (score % (N_CORES // 2)) * hs + (src_local % hs)
    assert src_rel.max() < wsize and src_rel.min() >= 0

    # per (core, window) in-degree over device-local ids
    qdeg = np.zeros((N_CORES, N_WIN, npc), dtype=np.int64)
    np.add.at(qdeg, (core_of, src_win, dst_local), 1)

    # per (core, window): node order by window in-degree desc
    node_at = np.empty((N_CORES, N_WIN, npc), dtype=np.int64)
    qpos = np.empty((N_CORES, N_WIN, npc), dtype=np.int64)
    for c in range(N_CORES):
        for q in range(N_WIN):
            o = np.argsort(-qdeg[c, q], kind="stable")
            node_at[c, q] = o
            qpos[c, q, o] = np.arange(npc)

    # shared tile degree profile per window (max across cores -> SPMD)
    D_q = np.zeros((N_WIN, tiles), dtype=np.int64)
    for q in range(N_WIN):
        sorted_deg = np.take_along_axis(qdeg[:, q, :], node_at[:, q, :], axis=1)
        D_q[q] = sorted_deg[:, ::P].max(axis=0)
    assert D_q.max() <= 96, f"window degree {D_q.max()} too large"

    # runs of equal D (D>0), split/packed into chunks
    win_chunks = []  # [q] -> list of (colbase, ccols, [(rt0,g,d,coloff)...])
    slots_q = np.zeros(N_WIN, dtype=np.int64)
    tile_run = np.full((N_WIN, tiles, 3), -1, dtype=np.int64)  # rt0, colstart, d
    for q in range(N_WIN):
        runs = []
        t0 = 0
        for tt in range(1, tiles + 1):
            if tt == tiles or D_q[q, tt] != D_q[q, t0]:
                if D_q[q, t0] > 0:
                    runs.append((t0, tt - t0, int(D_q[q, t0])))
                t0 = tt
        pieces = []
        for (rt0, g, d) in runs:
            max_g = max(1, CH // d)
            s = 0
            while s < g:
                gg = min(max_g, g - s)
                pieces.append((rt0 + s, gg, d))
                s += gg
        chunks = []
        cur, cur_cols = [], 0
        for pc in pieces:
            need = pc[1] * pc[2]
            assert need <= 96
            if cur and cur_cols + need > CH:
                chunks.append(cur)
                cur, cur_cols = [], 0
            cur.append(pc)
            cur_cols += need
        if cur:
            chunks.append(cur)
        # assign global columns
        col = 0
        out_chunks = []
        for chunk in chunks:
            cb = col
            rl = []
            for (rt0, g, d) in chunk:
                for k in range(g):
                    tile_run[q, rt0 + k] = (rt0, col, d)
                rl.append((rt0, g, d, col - cb))
                col += g * d
            out_chunks.append((cb, col - cb, rl))
        slots_q[q] = col
        win_chunks.append(out_chunks)

    # edge -> grid cell
    pos = qpos[core_of, src_win, dst_local]  # [E]
    t = pos // P
    pp = pos % P
    # rank j within (core, window, dst)
    key = (core_of * N_WIN + src_win) * npc + dst_local
    order = np.argsort(key, kind="stable")
    sk = key[order]
    first = np.flatnonzero(np.r_[True, sk[1:] != sk[:-1]])
    group_start = np.repeat(first, np.diff(np.r_[first, E]))
    j = np.empty(E, dtype=np.int64)
    j[order] = np.arange(E) - group_start
    rt0_e = tile_run[src_win, t, 0]
    colstart_e = tile_run[src_win, t, 1]
    d_e = tile_run[src_win, t, 2]
    assert (d_e > 0).all() and (j < d_e).all()
    # (e, g)-major within run: col = colstart_of_run_tile... per-tile colstart
    # holds the run's first column; offset = j*g + (t - rt0). g from run width:
    # store g per tile via tile_run? derive: g = run length; encode instead:
    # col = run_colstart + j*run_g + (t - rt0)
    # we stored per-tile (rt0, run_piece_colstart, d) for its PIECE; recompute
    # piece g: piece covers tiles [rt0, rt0+g) sharing same colstart.
    # Build per (q, tile): piece g
    piece_g = np.zeros((N_WIN, tiles), dtype=np.int64)
    for q in range(N_WIN):
        for (cb, ccols, rl) in win_chunks[q]:
            for (rt0, g, d, coff) in rl:
                piece_g[q, rt0 : rt0 + g] = g
    g_e = piece_g[src_win, t]
    col = colstart_e + j * g_e + (t - rt0_e)
    assert (col < slots_q[src_win]).all()

    # per-core gidx streams (concatenated over windows)
    wbase = np.concatenate([[0], np.cumsum(slots_q)]).astype(np.int64)
    tot_slots = int(slots_q.sum())
    gidx = np.empty((N_CORES, P, tot_slots * 8), dtype=np.int16)
    for c in range(N_CORES):
        for q in range(N_WIN):
            sq = int(slots_q[q])
            stream = np.full(sq * P, pad_rel, dtype=np.int64)
            m = (core_of == c) & (src_win == q)
            stream[col[m] * P + pp[m]] = src_rel[m]
            assert stream.max() < wsize and stream.min() >= 0
            gidx[c, :, int(wbase[q]) * 8 : int(wbase[q] + sq) * 8] = _wrap16(stream)

    # merge-gather index streams: acc row i (local order) <- partial_q[qpos[i]]
    mgidx = np.empty((N_CORES, P, N_WIN * npc // 16), dtype=np.int16)
    for c in range(N_CORES):
        for q in range(N_WIN):
            sl = slice(q * npc // 16, (q + 1) * npc // 16)
            mgidx[c, :, sl] = _wrap16(qpos[c, q])

    # max chunk width for SBUF sizing
    chw = max(
        max((cc for (_, cc, _) in win_chunks[q]), default=1) for q in range(N_WIN)
    )

    return dict(
        npc_raw=npc_raw, hs_raw=hs_raw, hs=hs, npc=npc, tiles=tiles,
        htiles=htiles, n_pad=n_pad, wsize=wsize, pad_rel=pad_rel,
        node_at=node_at, qpos=qpos, D_q=D_q, slots_q=slots_q, wbase=wbase,
        win_chunks=win_chunks, gidx=gidx, mgidx=mgidx, chw=chw,
        tot_slots=tot_slots,
    )


def _build_program(lay, f_in, hidden, ncls):
    tiles = lay["tiles"]
    htiles = lay["htiles"]
    npc = lay["npc"]
    hs = lay["hs"]
    n_pad = lay["n_pad"]
    wsize = lay["wsize"]
    slots_q = lay["slots_q"]
    wbase = lay["wbase"]
    win_chunks = lay["win_chunks"]
    chw = lay["chw"]
    tot_slots = lay["tot_slots"]
    F = 2 * ncls + 1
    assert F == f_in and ncls == 32

    nc = bacc.Bacc("TRN2", target_bir_lowering=False, debug=False,
                   enable_asserts=False, num_devices=N_CORES,
                   num_swdge_queues=4)

    xT_in = nc.dram_tensor("xT", [f_in, npc], FP32, kind="ExternalInput").ap()
    xqT_in = nc.dram_tensor("xqT", [f_in, N_WIN * npc], FP16,
                            kind="ExternalInput").ap()
    x_in = nc.dram_tensor("xrow", [npc, f_in], FP32, kind="ExternalInput").ap()
    wmlp_in = nc.dram_tensor("wmlp", [f_in, hidden], FP32, kind="ExternalInput").ap()
    bmlp_in = nc.dram_tensor("bmlp", [hidden, 1], FP32, kind="ExternalInput").ap()
    wcat_in = nc.dram_tensor("wcat", [hidden, HC], FP32, kind="ExternalInput").ap()
    wd_in = nc.dram_tensor("wd", [hidden, 2], FP16, kind="ExternalInput").ap()
    bb_in = nc.dram_tensor("bb", [P, 2 * ncls], FP32, kind="ExternalInput").ap()
    padm_in = nc.dram_tensor("padm", [P, tiles], FP16, kind="ExternalInput").ap()
    gidx_in = nc.dram_tensor("gidx", [P, tot_slots * 8], INT16,
                             kind="ExternalInput").ap()
    mgidx_in = nc.dram_tensor("mgidx", [P, N_WIN * npc // 16], INT16,
                              kind="ExternalInput").ap()
    out_t = nc.dram_tensor("out", [npc, F], FP32, kind="ExternalOutput").ap()

    qrr = [0]

    def next_q():
        q = qrr[0] % 4
        qrr[0] += 1
        return q

    with tile.TileContext(nc) as tc:
        with contextlib.ExitStack() as ctx:
            persist = ctx.enter_context(tc.tile_pool(name="persist", bufs=1))
            dram = ctx.enter_context(tc.tile_pool(name="dram", bufs=1, space="DRAM"))

            nc.gpsimd.load_library(library_config.mlp)

            x3buf = persist.tile([P, tiles], FP32)
            bb_sb = persist.tile([P, 2 * ncls], FP32)
            padm_sb = persist.tile([P, tiles], FP16)
            d12q = persist.tile([P, N_WIN, tiles, 2], FP16)
            acc = persist.tile([P, tiles, HC], FP16)
            nc.sync.dma_start(out=bb_sb[:], in_=bb_in[:])
            nc.sync.dma_start(out=padm_sb[:], in_=padm_in[:])

            hloc_d = dram.tile([npc, ELEM], FP16)
            htab_d = dram.tile([n_pad, ELEM], FP16)
            part_d = [dram.tile([npc, ELEM], FP16, name=f"part{q}", tag=f"part{q}")
                      for q in range(N_WIN)]

            # ---------------- Phase 1: dense local features ----------------
            GT = 4  # tiles per instruction group
            cpool = ctx.enter_context(tc.tile_pool(name="ph1c", bufs=1))
            wmlp_sb = cpool.tile([f_in, hidden], FP32)
            wmlp16 = cpool.tile([f_in, hidden], FP16)
            bmlp_sb = cpool.tile([hidden, 1], FP32)
            wcat_sb = cpool.tile([hidden, HC], FP32)
            wd_sb = cpool.tile([hidden, 2], FP16)
            nc.sync.dma_start(out=wmlp_sb[:], in_=wmlp_in[:])
            nc.sync.dma_start(out=bmlp_sb[:], in_=bmlp_in[:])
            nc.sync.dma_start(out=wcat_sb[:], in_=wcat_in[:])
            nc.sync.dma_start(out=wd_sb[:], in_=wd_in[:])
            nc.vector.tensor_copy(out=wmlp16[:], in_=wmlp_sb[:])
            ngrp = -(-tiles // GT)

            with tc.tile_pool(name="hl", bufs=1) as hlp, \
                 tc.tile_pool(name="ph1x", bufs=2) as ph1x, \
                 tc.tile_pool(name="ph1o", bufs=2) as ph1o, \
                 tc.tile_pool(name="psA", bufs=2, space="PSUM") as psAp, \
                 tc.tile_pool(name="psH", bufs=2, space="PSUM") as psHp, \
                 tc.tile_pool(name="psT", bufs=2, space="PSUM") as psTp:
                ident = hlp.tile([P, P], FP32)
                make_identity(nc, ident[:])

                hloc_sb = hlp.tile([P, tiles, HC], FP16)

                half_done = False
                for grp in range(ngrp):
                    t0 = grp * GT
                    gsz = min(GT, tiles - t0)
                    xt = ph1x.tile([f_in, GT * P], FP32, tag="xt")
                    nc.sync.dma_start(
                        out=xt[:, : gsz * P],
                        in_=xT_in[:, t0 * P : (t0 + gsz) * P],
                    )
                    psA = psAp.tile([P, GT * P], FP32, space="PSUM")
                    nc.tensor.matmul(
                        out=psA[:, : gsz * P], lhsT=wmlp_sb[:],
                        rhs=xt[:, : gsz * P], start=True, stop=True,
                    )
                    x0 = ph1o.tile([P, GT * P], FP32, tag="x0")
                    nc.scalar.activation(
                        out=x0[:, : gsz * P], in_=psA[:, : gsz * P],
                        func=mybir.ActivationFunctionType.Relu,
                        bias=bmlp_sb[:, 0:1], scale=1.0,
                    )
                    psH = psHp.tile([P, GT * HC], FP32, space="PSUM")
                    psT = psTp.tile([P, GT * P], FP32, space="PSUM")
                    for k in range(gsz):
                        nc.tensor.matmul(
                            out=psH[:, k * HC : (k + 1) * HC],
                            lhsT=x0[:, k * P : (k + 1) * P], rhs=wcat_sb[:],
                            start=True, stop=True,
                        )
                        nc.tensor.transpose(
                            out=psT[:, k * P : (k + 1) * P],
                            in_=x0[:, k * P : (k + 1) * P], identity=ident[:],
                        )
                    nc.vector.tensor_copy(
                        out=hloc_sb[:, t0 : t0 + gsz, :],
                        in_=psH[:, : gsz * HC].rearrange(
                            "p (t c) -> p t c", c=HC),
                    )
                    nc.vector.tensor_reduce(
                        out=x3buf[:, t0 : t0 + gsz],
                        in_=psT[:, : gsz * P].rearrange("p (t e) -> p t e", e=P),
                        axis=mybir.AxisListType.X, op=mybir.AluOpType.max,
                    )
                    if (not half_done) and (t0 + gsz) >= htiles:
                        # first half complete: mark pads, flush, AllGather 1
                        half_done = True
                        nc.vector.tensor_tensor(
                            out=hloc_sb[:, :htiles, S12 : S12 + 2],
                            in0=hloc_sb[:, :htiles, S12 : S12 + 2],
                            in1=padm_sb[:, :htiles].unsqueeze(2)
                            .broadcast_to([P, htiles, 2]),
                            op=mybir.AluOpType.add,
                        )
                        nc.sync.dma_start(
                            out=hloc_d[: hs, :HC].rearrange(
                                "(t p) c -> p t c", p=P),
                            in_=hloc_sb[:, :htiles, :],
                        )
                        nc.gpsimd.collective_compute(
                            "AllGather", mybir.AluOpType.bypass,
                            replica_groups=[list(range(N_CORES))],
                            ins=[hloc_d[0:hs, :].opt()],
                            outs=[htab_d[0 : N_CORES * hs, :].opt()],
                        )
                # second half: mark pads, flush, AllGather 2
                nc.vector.tensor_tensor(
                    out=hloc_sb[:, htiles:, S12 : S12 + 2],
                    in0=hloc_sb[:, htiles:, S12 : S12 + 2],
                    in1=padm_sb[:, htiles:].unsqueeze(2)
                    .broadcast_to([P, tiles - htiles, 2]),
                    op=mybir.AluOpType.add,
                )
                nc.sync.dma_start(
                    out=hloc_d[hs:, :HC].rearrange("(t p) c -> p t c", p=P),
                    in_=hloc_sb[:, htiles:, :],
                )
                nc.gpsimd.collective_compute(
                    "AllGather", mybir.AluOpType.bypass,
                    replica_groups=[list(range(N_CORES))],
                    ins=[hloc_d[hs:npc, :].opt()],
                    outs=[htab_d[N_CORES * hs :, :].opt()],
                )

            # ---------- d-term recompute per window (overlaps AG) ----------
            with tc.tile_pool(name="dqx", bufs=2) as dqx, \
                 tc.tile_pool(name="dqo", bufs=2) as dqo, \
                 tc.tile_pool(name="psA2", bufs=2, space="PSUM") as psA2p, \
                 tc.tile_pool(name="psD", bufs=2, space="PSUM") as psDp:
                for q in range(N_WIN):
                    for grp in range(ngrp):
                        t0 = grp * GT
                        gsz = min(GT, tiles - t0)
                        xq = dqx.tile([f_in, GT * P], FP16, tag="xq")
                        nc.sync.dma_start(
                            out=xq[:, : gsz * P],
                            in_=xqT_in[:, q * npc + t0 * P :
                                       q * npc + (t0 + gsz) * P],
                        )
                        psA2 = psA2p.tile([P, GT * P], FP32, space="PSUM")
                        nc.tensor.matmul(
                            out=psA2[:, : gsz * P], lhsT=wmlp16[:],
                            rhs=xq[:, : gsz * P], start=True, stop=True,
                        )
                        x0q = dqo.tile([P, GT * P], FP16, tag="x0q")
                        nc.scalar.activation(
                            out=x0q[:, : gsz * P], in_=psA2[:, : gsz * P],
                            func=mybir.ActivationFunctionType.Relu,
                            bias=bmlp_sb[:, 0:1], scale=1.0,
                        )
                        psD = psDp.tile([P, GT * 2], FP32, space="PSUM")
                        for k in range(gsz):
                            nc.tensor.matmul(
                                out=psD[:, 2 * k : 2 * k + 2],
                                lhsT=x0q[:, k * P : (k + 1) * P],
                                rhs=wd_sb[:], start=True, stop=True,
                            )
                        nc.vector.tensor_copy(
                            out=d12q[:, q, t0 : t0 + gsz, :],
                            in_=psD[:, : gsz * 2].rearrange(
                                "p (t c) -> p t c", c=2),
                        )

            # ---------------- Phase 3: per-window gather + reduce ----------
            MSG_BUFS = max(4, min(6, (96 * 1024) // (chw * 2 * ELEM)))
            with tc.tile_pool(name="gi", bufs=2) as gip, \
                 tc.tile_pool(name="mgi", bufs=1) as mgip, \
                 tc.tile_pool(name="msg", bufs=MSG_BUFS) as msgp, \
                 tc.tile_pool(name="exb", bufs=3) as exp_, \
                 tc.tile_pool(name="lrb", bufs=3) as lrp, \
                 tc.tile_pool(name="sc", bufs=3) as scp, \
                 tc.tile_pool(name="pt", bufs=2) as ptp:
                mgidx_sb = mgip.tile([P, N_WIN * npc // 16], INT16)
                nc.sync.dma_start(out=mgidx_sb[:], in_=mgidx_in[:])

                for q in range(N_WIN):
                    gw = gip.tile([P, int(slots_q.max()) * 8], INT16, tag="gw")
                    nc.sync.dma_start(
                        out=gw[:, : int(slots_q[q]) * 8],
                        in_=gidx_in[:, int(wbase[q]) * 8 :
                                    int(wbase[q] + slots_q[q]) * 8],
                    )
                    partial = ptp.tile([P, tiles, HC], FP16, tag="partial")
                    nc.vector.memset(partial[:], 0.0)

                    tab = htab_d[q * wsize : (q + 1) * wsize, :]
                    for (cb, ccols, rl) in win_chunks[q]:
                        msg = msgp.tile([P, chw, ELEM], FP16, tag="msg")
                        nc.gpsimd.dma_gather(
                            out_ap=msg[:, :ccols, :],
                            in_ap=tab,
                            idxs_ap=gw[:, cb * 8 : (cb + ccols) * 8],
                            num_idxs=ccols * P,
                            num_idxs_reg=ccols * P,
                            elem_size=ELEM,
                            single_packet=False,
                            queue_num=next_q(),
                        )
                        for (rt0, g, d, coff) in rl:
                            gd = g * d
                            mvS = msg[:, coff : coff + gd, S12 : S12 + 2] \
                                .rearrange("p (e g) c -> p g e c", g=g)
                            exb = exp_.tile([P, 2 * chw], FP16, tag="exb")
                            ex_gec = exb[:, : 2 * gd].rearrange(
                                "p (g e c) -> p g e c", e=d, c=2)
                            nc.vector.tensor_tensor(
                                out=ex_gec, in0=mvS,
                                in1=d12q[:, q, rt0 : rt0 + g, :]
                                .unsqueeze(2).broadcast_to([P, g, d, 2]),
                                op=mybir.AluOpType.add,
                            )
                            flat = exb[:, : 2 * gd]
                            lr = lrp.tile([P, 2 * chw], FP16, tag="lr")
                            nc.vector.tensor_scalar_mul(
                                lr[:, : 2 * gd], flat, NEG_SLOPE)
                            nc.vector.tensor_tensor(
                                out=flat, in0=flat, in1=lr[:, : 2 * gd],
                                op=mybir.AluOpType.max,
                            )
                            nc.scalar.activation(
                                out=flat, in_=flat,
                                func=mybir.ActivationFunctionType.Exp,
                            )
                            with nc.allow_low_precision("fp16 partials"):
                                for conv in range(2):
                                    hb = conv * 32
                                    sct = scp.tile([P, 32 * chw], FP16,
                                                   tag="sct")
                                    out_gce = sct[:, : 32 * gd].rearrange(
                                        "p (g c e) -> p g c e", c=32, e=d)
                                    mv_h = msg[:, coff : coff + gd,
                                               hb : hb + 32].rearrange(
                                        "p (e g) c -> p g c e", g=g)
                                    ex_bc = exb[:, : 2 * gd].rearrange(
                                        "p (g e c) -> p g e c", e=d, c=2
                                    )[:, :, :, conv].unsqueeze(2) \
                                        .broadcast_to([P, g, 32, d])
                                    nc.vector.tensor_tensor(
                                        out=out_gce, in0=mv_h, in1=ex_bc,
                                        op=mybir.AluOpType.mult,
                                    )
                                    nc.vector.tensor_reduce(
                                        out=partial[:, rt0 : rt0 + g,
                                                    conv * 32 : conv * 32 + 32],
                                        in_=sct[:, : 32 * gd].rearrange(
                                            "p (g c e) -> p g c e", c=32, e=d),
                                        axis=mybir.AxisListType.X,
                                        op=mybir.AluOpType.add,
                                    )
                                nc.vector.tensor_reduce(
                                    out=partial[:, rt0 : rt0 + g, S12 : S12 + 2],
                                    in_=exb[:, : 2 * gd].rearrange(
                                        "p (g e c) -> p g c e", e=d, c=2),
                                    axis=mybir.AxisListType.X,
                                    op=mybir.AluOpType.add,
                                )
                    nc.sync.dma_start(
                        out=part_d[q][:, :HC].rearrange("(t p) c -> p t c", p=P),
                        in_=partial[:],
                    )
                    # merge this window's partial into acc (local node order)
                    t_half = tiles // 2
                    for hi, (tb, te) in enumerate(((0, t_half), (t_half, tiles))):
                        nidx = (te - tb) * P
                        mgt = msgp.tile([P, chw, ELEM], FP16, tag="msg")
                        nteff = te - tb
                        assert nteff * ELEM <= chw * ELEM
                        ib = q * npc // 16 + tb * P // 16
                        nc.gpsimd.dma_gather(
                            out_ap=mgt[:, :nteff, :],
                            in_ap=part_d[q][:],
                            idxs_ap=mgidx_sb[:, ib : ib + nidx // 16],
                            num_idxs=nidx,
                            num_idxs_reg=nidx,
                            elem_size=ELEM,
                            single_packet=False,
                            queue_num=next_q(),
                        )
                        if q == 0:
                            nc.vector.tensor_copy(
                                out=acc[:, tb:te, :], in_=mgt[:, :nteff, :HC])
                        else:
                            nc.vector.tensor_tensor(
                                out=acc[:, tb:te, :], in0=acc[:, tb:te, :],
                                in1=mgt[:, :nteff, :HC],
                                op=mybir.AluOpType.add,
                            )

            # ------------- Phase 4: normalize + residual + lsm -------------
            with tc.tile_pool(name="fin", bufs=1) as finp, \
                 tc.tile_pool(name="tmp", bufs=1) as tmpp:
                xin = finp.tile([P, tiles, F], FP32)
                nc.sync.dma_start(
                    out=xin[:], in_=x_in[:].rearrange("(t p) f -> p t f", p=P)
                )
                den32 = tmpp.tile([P, tiles], FP32, tag="den32")
                rden = tmpp.tile([P, tiles], FP32, tag="rden")
                numf = tmpp.tile([P, tiles, ncls], FP32, tag="numf")
                for conv in range(2):
                    nc.vector.tensor_copy(
                        out=den32[:], in_=acc[:, :, S12 + conv])
                    nc.vector.tensor_scalar_add(den32[:], den32[:], 1e-16)
                    nc.vector.reciprocal(out=rden[:], in_=den32[:])
                    nc.vector.tensor_tensor(
                        out=numf[:], in0=acc[:, :, conv * 32 : conv * 32 + 32],
                        in1=rden[:].unsqueeze(2).broadcast_to([P, tiles, ncls]),
                        op=mybir.AluOpType.mult,
                    )
                    nc.vector.tensor_tensor(
                        out=numf[:], in0=numf[:],
                        in1=bb_sb[:, conv * ncls : (conv + 1) * ncls]
                        .unsqueeze(1).broadcast_to([P, tiles, ncls]),
                        op=mybir.AluOpType.add,
                    )
                    if conv == 0:
                        nc.vector.tensor_scalar_max(numf[:], numf[:], 0.0)
                    nc.vector.tensor_tensor(
                        out=xin[:, :, conv * ncls : (conv + 1) * ncls],
                        in0=xin[:, :, conv * ncls : (conv + 1) * ncls],
                        in1=numf[:],
                        op=mybir.AluOpType.add,
                    )
                nc.vector.tensor_tensor(
                    out=xin[:, :, 2 * ncls], in0=xin[:, :, 2 * ncls],
                    in1=x3buf[:], op=mybir.AluOpType.add,
                )
                mx = tmpp.tile([P, tiles], FP32, tag="mx")
                nc.vector.tensor_reduce(
                    out=mx[:], in_=xin[:], axis=mybir.AxisListType.X,
                    op=mybir.AluOpType.max,
                )
                nc.vector.tensor_tensor(
                    out=xin[:], in0=xin[:],
                    in1=mx[:].unsqueeze(2).broadcast_to([P, tiles, F]),
                    op=mybir.AluOpType.subtract,
                )
                et = tmpp.tile([P, tiles, F], FP32, tag="et")
                nc.scalar.activation(
                    out=et[:], in_=xin[:],
                    func=mybir.ActivationFunctionType.Exp,
                )
                sm = tmpp.tile([P, tiles], FP32, tag="sm")
                nc.vector.tensor_reduce(
                    out=sm[:], in_=et[:], axis=mybir.AxisListType.X,
                    op=mybir.AluOpType.add,
                )
                lg = tmpp.tile([P, tiles], FP32, tag="lg")
                nc.scalar.activation(
                    out=lg[:], in_=sm[:],
                    func=mybir.ActivationFunctionType.Ln,
                )
                nc.vector.tensor_tensor(
                    out=xin[:], in0=xin[:],
                    in1=lg[:].unsqueeze(2).broadcast_to([P, tiles, F]),
                    op=mybir.AluOpType.subtract,
                )
                nc.sync.dma_start(
                    out=out_t[:].rearrange("(t p) f -> p t f", p=P), in_=xin[:]
                )

    nc.compile()
    return nc


def _run(nc, lay, x, W_mlp, b_mlp, W1, a1_src, a1_dst, b1,
         W2, a2_src, a2_dst, b2, trace=False):
    n_nodes, f_in = x.shape
    hidden = W_mlp.shape[1]
    ncls = W1.shape[1]
    npc = lay["npc"]
    npc_raw = lay["npc_raw"]
    hs_raw = lay["hs_raw"]
    hs = lay["hs"]
    tiles = lay["tiles"]

    x = np.asarray(x, dtype=np.float32)

    wcat = np.concatenate(
        [W1, W2, (W1 @ a1_src)[:, None], (W2 @ a2_src)[:, None]], axis=1
    ).astype(np.float32)
    assert wcat.shape == (hidden, HC)
    wd = np.stack([W1 @ a1_dst, W2 @ a2_dst], axis=1).astype(np.float16)
    bb = np.broadcast_to(
        np.concatenate([b1, b2])[None, :], (P, 2 * ncls)
    ).astype(np.float32).copy()

    padm = np.zeros((npc,), dtype=np.float16)
    padm[hs_raw:hs] = DUMMY_S
    padm[hs + hs_raw :] = DUMMY_S
    padm = np.ascontiguousarray(padm.reshape(tiles, P).T)

    in_maps = []
    for c in range(N_CORES):
        lo = c * npc_raw
        hi = min(lo + npc_raw, n_nodes)
        nreal = hi - lo
        n0 = min(hs_raw, nreal)
        xp = np.zeros((npc, f_in), dtype=np.float32)
        xp[:n0] = x[lo : lo + n0]
        xp[hs : hs + (nreal - n0)] = x[lo + n0 : hi]
        xq = np.empty((N_WIN, f_in, npc), dtype=np.float16)
        for q in range(N_WIN):
            xq[q] = xp[lay["node_at"][c, q]].T
        in_maps.append({
            "xT": np.ascontiguousarray(xp.T),
            "xqT": np.ascontiguousarray(np.concatenate(list(xq), axis=1)),
            "xrow": xp,
            "wmlp": np.asarray(W_mlp, dtype=np.float32),
            "bmlp": np.asarray(b_mlp, dtype=np.float32)[:, None].copy(),
            "wcat": wcat,
            "wd": wd,
            "bb": bb,
            "padm": padm,
            "gidx": np.ascontiguousarray(lay["gidx"][c]),
            "mgidx": np.ascontiguousarray(lay["mgidx"][c]),
        })

    res = bass_utils.run_bass_kernel_spmd(
        nc, in_maps, core_ids=list(range(N_CORES)), trace=trace
    )
    final = np.empty((n_nodes, f_in), dtype=np.float32)
    for c in range(N_CORES):
        lo = c * npc_raw
        hi = min(lo + npc_raw, n_nodes)
        nreal = hi - lo
        n0 = min(hs_raw, nreal)
        o = res.results[c]["out"]
        final[lo : lo + n0] = o[:n0]
        final[lo + n0 : hi] = o[hs : hs + (nreal - n0)]
    return final, res


def kernel(x, edge_index, W_mlp, b_mlp, W1, a1_src, a1_dst, b1,
           W2, a2_src, a2_dst, b2, trace=False, _ret_res=False):
    x = np.asarray(x)
    lay = _build_layout(edge_index, x.shape[0])
    nc = _build_program(lay, x.shape[1], W_mlp.shape[1], W1.shape[1])
    out, res = _run(nc, lay, x, W_mlp, b_mlp, W1, a1_src, a1_dst, b1,
                    W2, a2_src, a2_dst, b2, trace=trace)
    if _ret_res:
        return out, res
    return out


# revision 7
# speedup vs baseline: 1.8769x; 1.0665x over previous
"""GAT-style 2-conv GNN forward on 8 Trainium2 NeuronCores (v3).

Strategy (graph/data parallel):
  - Nodes partitioned across 8 cores by destination range. Each core computes
    dense per-node features (packed 66-col fp16 row [h1|h2|s1|s2]); the table
    is replicated via two half AllGathers. Table rows are grouped [all cores'
    first halves | all cores' second halves] so each 25088-row
    int16-addressable window is completed by one half-AllGather; the second
    AllGather is issued after window-1 processing (windows 2-3 consume it).
  - Per-edge gathers use InstDMAGatherAnt round-robined over 4 SWDGE queues
    (descriptor generation is ~7.8ns/idx per queue; 4 queues overlap).
  - Per (core, window), destination nodes are degree-sorted into a dense
    layer-major (ELL-transposed) slot grid: layer k holds the k-th incoming
    edge of every node with degree > k. Layers are contiguous tile prefixes,
    so the segmented softmax-weighted reduction is a short sequence of flat
    fp16 tensor ops per 64-column chunk (measured-fast DVE access patterns).
  - Attention d-terms per window order are recomputed from host-permuted x
    via TensorEngine matmuls (no gather); per-slot d comes from per-layer
    prefix copies.
  - Window partials are merged back to local node order with one small
    gather per window half, interleaved into the next window's gather stream
    to avoid head-of-line blocking on the descriptor engine.
  - Softmax max-subtraction is skipped (logits are O(1); exp cannot
    overflow in fp16; algebraically identical).
  - Final: out = log_softmax([x1 | x2 | x3] + x) per node tile.

Host work is index/layout preprocessing only (sort, pad, permute, weight
concatenation); all network floating-point math runs on device.
"""

import sys

sys.path.insert(0, "/opt/trn_rl_repo")

import contextlib

import numpy as np

import concourse.bacc as bacc
import concourse.bass_utils as bass_utils
import concourse.mybir as mybir
import concourse.tile as tile
from concourse import library_config
from concourse.masks import make_identity

FP32 = mybir.dt.float32
FP16 = mybir.dt.float16
INT16 = mybir.dt.int16

N_CORES = 8
N_WIN = 4
P = 128
ELEM = 128  # fp16 cols per table row = 256B gather element
HC = 66  # used table cols [h1(32) | h2(32) | s1 | s2]
S12 = 64
NEG_SLOPE = 0.2
DUMMY_S = -30000.0
CH = 64  # chunk columns (num_idxs = 8192 <= 12288 HW limit)


def _wrap16(stream):
    """[n] -> [128, n//16] int16 in the 16-partition wrapped+replicated layout."""
    n = stream.shape[0]
    assert n % 16 == 0
    w = stream.reshape(n // 16, 16).T.astype(np.int16)
    return np.tile(w, (8, 1))


def _build_layout(edge_index, n_nodes):
    src = np.asarray(edge_index[0], dtype=np.int64)
    dst = np.asarray(edge_index[1], dtype=np.int64)
    E = src.shape[0]

    npc_raw = -(-n_nodes // N_CORES)  # real nodes per core
    hs_raw = -(-npc_raw // 2)  # real nodes per half
    hs = -(-hs_raw // P) * P
    if hs == hs_raw:  # ensure pad rows exist in each half
        hs += P
    npc = 2 * hs
    tiles = npc // P
    htiles = hs // P
    n_pad = npc * N_CORES
    wsize = (N_CORES // 2) * hs
    assert wsize <= 32768
    pad_rel = hs_raw  # first pad row of a window's first core-slot

    core_of = np.minimum(dst // npc_raw, N_CORES - 1)
    score = np.minimum(src // npc_raw, N_CORES - 1)

    def to_local(orig_local):
        return np.where(orig_local < hs_raw, orig_local, hs + orig_local - hs_raw)

    dst_local = to_local(dst - core_of * npc_raw)
    src_local = to_local(src - score * npc_raw)

    src_half = (src_local >= hs).astype(np.int64)
    src_win = 2 * src_half + (score // (N_CORES // 2))
    src_rel = (score % (N_CORES // 2)) * hs + (src_local % hs)
    assert src_rel.max() < wsize and src_rel.min() >= 0

    # per (core, window) in-degree over device-local ids
    qdeg = np.zeros((N_CORES, N_WIN, npc), dtype=np.int64)
    np.add.at(qdeg, (core_of, src_win, dst_local), 1)

    node_at = np.empty((N_CORES, N_WIN, npc), dtype=np.int64)
    qpos = np.empty((N_CORES, N_WIN, npc), dtype=np.int64)
    for c in range(N_CORES):
        for q in range(N_WIN):
            o = np.argsort(-qdeg[c, q], kind="stable")
            node_at[c, q] = o
            qpos[c, q, o] = np.arange(npc)

    # shared tile degree profile per window (max across cores -> SPMD)
    D_q = np.zeros((N_WIN, tiles), dtype=np.int64)
    for q in range(N_WIN):
        sorted_deg = np.take_along_axis(qdeg[:, q, :], node_at[:, q, :], axis=1)
        D_q[q] = sorted_deg[:, ::P].max(axis=0)
    assert D_q.max() <= 96, f"window degree {D_q.max()} too large"

    # layer-major slot grid: layer k = tiles [0, n_k) with D > k
    win_layers = []  # [q] -> list of (Lk, nk)
    win_chunks = []  # [q] -> list of (cb, ccols, [(sa, scount, ta)...])
    slots_q = np.zeros(N_WIN, dtype=np.int64)
    for q in range(N_WIN):
        maxd = int(D_q[q].max())
        layers = []
        col = 0
        for k in range(maxd):
            nk = int((D_q[q] > k).sum())
            assert nk > 0
            layers.append((col, nk))
            col += nk
        slots_q[q] = col
        win_layers.append(layers)
        # chunks of CH columns; segments at layer boundaries
        chunks = []
        cb = 0
        while cb < col:
            cc = min(CH, col - cb)
            segs = []
            for k, (Lk, nk) in enumerate(layers):
                a = max(cb, Lk)
                b = min(cb + cc, Lk + nk)
                if b > a:
                    segs.append((a - cb, b - a, a - Lk))
            chunks.append((cb, cc, segs))
            cb += cc
        win_chunks.append(chunks)

    # edge -> grid cell: col = L_base[rank] + tile
    pos = qpos[core_of, src_win, dst_local]
    t = pos // P
    pp = pos % P
    key = (core_of * N_WIN + src_win) * npc + dst_local
    order = np.argsort(key, kind="stable")
    sk = key[order]
    first = np.flatnonzero(np.r_[True, sk[1:] != sk[:-1]])
    group_start = np.repeat(first, np.diff(np.r_[first, E]))
    j = np.empty(E, dtype=np.int64)
    j[order] = np.arange(E) - group_start
    assert (j < D_q[src_win, t]).all()
    lbase_all = np.zeros((N_WIN, int(D_q.max()) + 1), dtype=np.int64)
    for q in range(N_WIN):
        for k, (Lk, nk) in enumerate(win_layers[q]):
            lbase_all[q, k] = Lk
    col = lbase_all[src_win, j] + t
    assert (col < slots_q[src_win]).all()

    wbase = np.concatenate([[0], np.cumsum(slots_q)]).astype(np.int64)
    tot_slots = int(slots_q.sum())
    gidx = np.empty((N_CORES, P, tot_slots * 8), dtype=np.int16)
    for c in range(N_CORES):
        for q in range(N_WIN):
            sq = int(slots_q[q])
            stream = np.full(sq * P, pad_rel, dtype=np.int64)
            m = (core_of == c) & (src_win == q)
            stream[col[m] * P + pp[m]] = src_rel[m]
            assert stream.max() < wsize and stream.min() >= 0
            gidx[c, :, int(wbase[q]) * 8 : int(wbase[q] + sq) * 8] = _wrap16(stream)

    # merge-gather index streams: acc row i (local order) <- partial_q[qpos[i]]
    mgidx = np.empty((N_CORES, P, N_WIN * npc // 16), dtype=np.int16)
    for c in range(N_CORES):
        for q in range(N_WIN):
            sl = slice(q * npc // 16, (q + 1) * npc // 16)
            mgidx[c, :, sl] = _wrap16(qpos[c, q])

    return dict(
        npc_raw=npc_raw, hs_raw=hs_raw, hs=hs, npc=npc, tiles=tiles,
        htiles=htiles, n_pad=n_pad, wsize=wsize, pad_rel=pad_rel,
        node_at=node_at, qpos=qpos, D_q=D_q, slots_q=slots_q, wbase=wbase,
        win_layers=win_layers, win_chunks=win_chunks, gidx=gidx, mgidx=mgidx,
        tot_slots=tot_slots,
    )


def _build_program(lay, f_in, hidden, ncls):
    tiles = lay["tiles"]
    htiles = lay["htiles"]
    npc = lay["npc"]
    hs = lay["hs"]
    n_pad = lay["n_pad"]
    wsize = lay["wsize"]
    slots_q = lay["slots_q"]
    wbase = lay["wbase"]
    win_layers = lay["win_layers"]
    win_chunks = lay["win_chunks"]
    tot_slots = lay["tot_slots"]
    smax = int(slots_q.max())
    F = 2 * ncls + 1
    assert F == f_in and ncls == 32

    nc = bacc.Bacc("TRN2", target_bir_lowering=False, debug=False,
                   enable_asserts=False, num_devices=N_CORES,
                   num_swdge_queues=4)

    xT_in = nc.dram_tensor("xT", [f_in, npc], FP16, kind="ExternalInput").ap()
    xqT_in = nc.dram_tensor("xqT", [f_in, N_WIN * npc], FP16,
                            kind="ExternalInput").ap()
    x_in = nc.dram_tensor("xrow", [npc, f_in], FP32, kind="ExternalInput").ap()
    wmlp_in = nc.dram_tensor("wmlp", [f_in, hidden], FP16, kind="ExternalInput").ap()
    bmlp_in = nc.dram_tensor("bmlp", [hidden, 1], FP32, kind="ExternalInput").ap()
    wcat_in = nc.dram_tensor("wcat", [hidden, HC], FP16, kind="ExternalInput").ap()
    wd_in = nc.dram_tensor("wd", [hidden, 2], FP16, kind="ExternalInput").ap()
    bb_in = nc.dram_tensor("bb", [P, 2 * ncls], FP32, kind="ExternalInput").ap()
    padm_in = nc.dram_tensor("padm", [P, tiles], FP16, kind="ExternalInput").ap()
    gidx_in = nc.dram_tensor("gidx", [P, tot_slots * 8], INT16,
                             kind="ExternalInput").ap()
    mgidx_in = nc.dram_tensor("mgidx", [P, N_WIN * npc // 16], INT16,
                              kind="ExternalInput").ap()
    out_t = nc.dram_tensor("out", [npc, F], FP32, kind="ExternalOutput").ap()

    qrr = [0]

    def next_q():
        q = qrr[0] % 4
        qrr[0] += 1
        return q

    with tile.TileContext(nc) as tc:
        with contextlib.ExitStack() as ctx:
            persist = ctx.enter_context(tc.tile_pool(name="persist", bufs=1))
            dram = ctx.enter_context(tc.tile_pool(name="dram", bufs=1, space="DRAM"))

            nc.gpsimd.load_library(library_config.mlp)

            x3buf = persist.tile([P, tiles], FP32)
            bb_sb = persist.tile([P, 2 * ncls], FP32)
            padm_sb = persist.tile([P, tiles], FP16)
            d12T = persist.tile([P, N_WIN, 2, tiles], FP16)
            acc = persist.tile([P, tiles, HC], FP16)
            nc.sync.dma_start(out=bb_sb[:], in_=bb_in[:])
            nc.sync.dma_start(out=padm_sb[:], in_=padm_in[:])

            hloc_d = dram.tile([npc, ELEM], FP16)
            htab_lo = dram.tile([N_CORES * hs, ELEM], FP16, name="htl", tag="htl")
            htab_hi = dram.tile([N_CORES * hs, ELEM], FP16, name="hth", tag="hth")
            part_d = [dram.tile([npc, ELEM], FP16, name=f"part{q}", tag=f"part{q}")
                      for q in range(N_WIN)]

            cpool = ctx.enter_context(tc.tile_pool(name="consts", bufs=1))
            wmlp_sb = cpool.tile([f_in, hidden], FP16)
            bmlp_sb = cpool.tile([hidden, 1], FP32)
            wcat_sb = cpool.tile([hidden, HC], FP16)
            wd_sb = cpool.tile([hidden, 2], FP16)
            nc.sync.dma_start(out=wmlp_sb[:], in_=wmlp_in[:])
            nc.sync.dma_start(out=bmlp_sb[:], in_=bmlp_in[:])
            nc.sync.dma_start(out=wcat_sb[:], in_=wcat_in[:])
            nc.sync.dma_start(out=wd_sb[:], in_=wd_in[:])

            # ---------------- Phase 1: dense local features ----------------
            GT = 4
            ngrp = -(-tiles // GT)
            with tc.tile_pool(name="hl", bufs=1) as hlp, \
                 tc.tile_pool(name="ph1x", bufs=3) as ph1x, \
                 tc.tile_pool(name="ph1o", bufs=2) as ph1o, \
                 tc.tile_pool(name="psA", bufs=2, space="PSUM") as psAp, \
                 tc.tile_pool(name="psH", bufs=2, space="PSUM") as psHp, \
                 tc.tile_pool(name="psT", bufs=2, space="PSUM") as psTp:
                ident = hlp.tile([P, P], FP16)
                make_identity(nc, ident[:])
                hloc_sb = hlp.tile([P, tiles, HC], FP16)

                half_done = False
                for grp in range(ngrp):
                    t0 = grp * GT
                    gsz = min(GT, tiles - t0)
                    xt = ph1x.tile([f_in, GT * P], FP16, tag="xt")
                    nc.sync.dma_start(
                        out=xt[:, : gsz * P],
                        in_=xT_in[:, t0 * P : (t0 + gsz) * P],
                    )
                    psA = psAp.tile([P, GT * P], FP32, space="PSUM")
                    nc.tensor.matmul(
                        out=psA[:, : gsz * P], lhsT=wmlp_sb[:],
                        rhs=xt[:, : gsz * P], start=True, stop=True,
                    )
                    x0 = ph1o.tile([P, GT * P], FP16, tag="x0")
                    nc.scalar.activation(
                        out=x0[:, : gsz * P], in_=psA[:, : gsz * P],
                        func=mybir.ActivationFunctionType.Relu,
                        bias=bmlp_sb[:, 0:1], scale=1.0,
                    )
                    psH = psHp.tile([P, GT * HC], FP32, space="PSUM")
                    psT = psTp.tile([P, GT * P], FP16, space="PSUM")
                    for k in range(gsz):
                        nc.tensor.matmul(
                            out=psH[:, k * HC : (k + 1) * HC],
                            lhsT=x0[:, k * P : (k + 1) * P], rhs=wcat_sb[:],
                            start=True, stop=True,
                        )
                        nc.tensor.transpose(
                            out=psT[:, k * P : (k + 1) * P],
                            in_=x0[:, k * P : (k + 1) * P], identity=ident[:],
                        )
                    nc.vector.tensor_copy(
                        out=hloc_sb[:, t0 : t0 + gsz, :],
                        in_=psH[:, : gsz * HC].rearrange(
                            "p (t c) -> p t c", c=HC),
                    )
                    nc.vector.tensor_reduce(
                        out=x3buf[:, t0 : t0 + gsz],
                        in_=psT[:, : gsz * P].rearrange("p (t e) -> p t e", e=P),
                        axis=mybir.AxisListType.X, op=mybir.AluOpType.max,
                    )
                    if (not half_done) and (t0 + gsz) >= htiles:
                        half_done = True
                        nc.vector.tensor_tensor(
                            out=hloc_sb[:, :htiles, S12 : S12 + 2],
                            in0=hloc_sb[:, :htiles, S12 : S12 + 2],
                            in1=padm_sb[:, :htiles].unsqueeze(2)
                            .broadcast_to([P, htiles, 2]),
                            op=mybir.AluOpType.add,
                        )
                        nc.sync.dma_start(
                            out=hloc_d[:hs, :HC].rearrange(
                                "(t p) c -> p t c", p=P),
                            in_=hloc_sb[:, :htiles, :],
                        )
                        nc.gpsimd.collective_compute(
                            "AllGather", mybir.AluOpType.bypass,
                            replica_groups=[list(range(N_CORES))],
                            ins=[hloc_d[0:hs, :].opt()],
                            outs=[htab_lo[:].opt()],
                        )
                nc.vector.tensor_tensor(
                    out=hloc_sb[:, htiles:, S12 : S12 + 2],
                    in0=hloc_sb[:, htiles:, S12 : S12 + 2],
                    in1=padm_sb[:, htiles:].unsqueeze(2)
                    .broadcast_to([P, tiles - htiles, 2]),
                    op=mybir.AluOpType.add,
                )
                nc.sync.dma_start(
                    out=hloc_d[hs:, :HC].rearrange("(t p) c -> p t c", p=P),
                    in_=hloc_sb[:, htiles:, :],
                )

            # ---------- d-term recompute per window (overlaps AG1) ---------
            with tc.tile_pool(name="dqx", bufs=3) as dqx, \
                 tc.tile_pool(name="dqo", bufs=2) as dqo, \
                 tc.tile_pool(name="psA2", bufs=2, space="PSUM") as psA2p, \
                 tc.tile_pool(name="psD", bufs=2, space="PSUM") as psDp:
                for q in range(N_WIN):
                    for grp in range(ngrp):
                        t0 = grp * GT
                        gsz = min(GT, tiles - t0)
                        xq = dqx.tile([f_in, GT * P], FP16, tag="xq")
                        nc.sync.dma_start(
                            out=xq[:, : gsz * P],
                            in_=xqT_in[:, q * npc + t0 * P :
                                       q * npc + (t0 + gsz) * P],
                        )
                        psA2 = psA2p.tile([P, GT * P], FP32, space="PSUM")
                        nc.tensor.matmul(
                            out=psA2[:, : gsz * P], lhsT=wmlp_sb[:],
                            rhs=xq[:, : gsz * P], start=True, stop=True,
                        )
                        x0q = dqo.tile([P, GT * P], FP16, tag="x0q")
                        nc.scalar.activation(
                            out=x0q[:, : gsz * P], in_=psA2[:, : gsz * P],
                            func=mybir.ActivationFunctionType.Relu,
                            bias=bmlp_sb[:, 0:1], scale=1.0,
                        )
                        psD = psDp.tile([P, GT * 2], FP32, space="PSUM")
                        for k in range(gsz):
                            nc.tensor.matmul(
                                out=psD[:, 2 * k : 2 * k + 2],
                                lhsT=x0q[:, k * P : (k + 1) * P],
                                rhs=wd_sb[:], start=True, stop=True,
                            )
                        nc.vector.tensor_copy(
                            out=d12T[:, q, :, t0 : t0 + gsz],
                            in_=psD[:, : gsz * 2].rearrange(
                                "p (t v) -> p v t", v=2),
                        )

            # ---------------- Phase 3: per-window gather + reduce ----------
            with tc.tile_pool(name="gi", bufs=2) as gip, \
                 tc.tile_pool(name="mgi", bufs=1) as mgip, \
                 tc.tile_pool(name="msg", bufs=6) as msgp, \
                 tc.tile_pool(name="exw", bufs=2) as exwp, \
                 tc.tile_pool(name="dxp", bufs=2) as dxpp, \
                 tc.tile_pool(name="lrb", bufs=3) as lrp, \
                 tc.tile_pool(name="sc", bufs=3) as scp, \
                 tc.tile_pool(name="den", bufs=2) as denp, \
                 tc.tile_pool(name="pt", bufs=2) as ptp:
                mgidx_sb = mgip.tile([P, N_WIN * npc // 16], INT16)
                nc.sync.dma_start(out=mgidx_sb[:], in_=mgidx_in[:])

                pending_mg = [None]

                def emit_mg(q):
                    t_half = tiles // 2
                    for (tb, te) in ((0, t_half), (t_half, tiles)):
                        nidx = (te - tb) * P
                        nteff = te - tb
                        mgt = msgp.tile([P, CH, ELEM], FP16, tag="msg")
                        assert nteff <= CH
                        ib = q * npc // 16 + tb * P // 16
                        nc.gpsimd.dma_gather(
                            out_ap=mgt[:, :nteff, :],
                            in_ap=part_d[q][:],
                            idxs_ap=mgidx_sb[:, ib : ib + nidx // 16],
                            num_idxs=nidx,
                            num_idxs_reg=nidx,
                            elem_size=ELEM,
                            single_packet=False,
                            queue_num=next_q(),
                        )
                        if q == 0:
                            nc.vector.tensor_copy(
                                out=acc[:, tb:te, :], in_=mgt[:, :nteff, :HC])
                        else:
                            nc.vector.tensor_tensor(
                                out=acc[:, tb:te, :], in0=acc[:, tb:te, :],
                                in1=mgt[:, :nteff, :HC],
                                op=mybir.AluOpType.add,
                            )

                for q in range(N_WIN):
                    sq = int(slots_q[q])
                    tab = (htab_lo if q < 2 else htab_hi)[
                        (q % 2) * wsize : (q % 2 + 1) * wsize, :]
                    gw = gip.tile([P, smax * 8], INT16, tag="gw")
                    nc.sync.dma_start(
                        out=gw[:, : sq * 8],
                        in_=gidx_in[:, int(wbase[q]) * 8 :
                                    int(wbase[q] + slots_q[q]) * 8],
                    )
                    dexp = dxpp.tile([P, 2, smax], FP16, tag="dexp")
                    for (Lk, nk) in win_layers[q]:
                        nc.vector.tensor_copy(
                            out=dexp[:, :, Lk : Lk + nk],
                            in_=d12T[:, q, :, :nk],
                        )
                    partial = ptp.tile([P, tiles, HC], FP16, tag="partial")
                    nc.vector.memset(partial[:], 0.0)
                    den = denp.tile([P, 2, tiles], FP16, tag="den")
                    nc.vector.memset(den[:], 0.0)
                    exw = exwp.tile([P, 2, smax], FP16, tag="exw")

                    for ci, (cb, cc, segs) in enumerate(win_chunks[q]):
                        if ci == 2 and pending_mg[0] is not None:
                            emit_mg(pending_mg[0])
                            pending_mg[0] = None
                        msg = msgp.tile([P, CH, ELEM], FP16, tag="msg")
                        nc.gpsimd.dma_gather(
                            out_ap=msg[:, :cc, :],
                            in_ap=tab,
                            idxs_ap=gw[:, cb * 8 : (cb + cc) * 8],
                            num_idxs=cc * P,
                            num_idxs_reg=cc * P,
                            elem_size=ELEM,
                            single_packet=False,
                            queue_num=next_q(),
                        )
                        exs = exw[:, :, cb : cb + cc]
                        for cv in range(2):
                            nc.vector.tensor_tensor(
                                out=exs[:, cv, :],
                                in0=msg[:, :cc, S12 + cv],
                                in1=dexp[:, cv, cb : cb + cc],
                                op=mybir.AluOpType.add,
                            )
                        lr = lrp.tile([P, 2, CH], FP16, tag="lr")
                        nc.vector.tensor_scalar_mul(
                            lr[:, :, :cc], exs, NEG_SLOPE)
                        nc.vector.tensor_tensor(
                            out=exs, in0=exs, in1=lr[:, :, :cc],
                            op=mybir.AluOpType.max,
                        )
                        nc.scalar.activation(
                            out=exs, in_=exs,
                            func=mybir.ActivationFunctionType.Exp,
                        )
                        sct = scp.tile([P, CH, 2 * ncls], FP16, tag="sct")
                        nc.vector.tensor_tensor(
                            out=sct[:, :cc, :].rearrange(
                                "p s (v c) -> p s v c", v=2),
                            in0=msg[:, :cc, : 2 * ncls].rearrange(
                                "p s (v c) -> p s v c", v=2),
                            in1=exs.rearrange("p v s -> p s v").unsqueeze(3)
                            .broadcast_to([P, cc, 2, ncls]),
                            op=mybir.AluOpType.mult,
                        )
                        with nc.allow_low_precision("fp16 partials"):
                            for (sa, scount, ta) in segs:
                                nc.vector.tensor_tensor(
                                    out=partial[:, ta : ta + scount, : 2 * ncls],
                                    in0=partial[:, ta : ta + scount, : 2 * ncls],
                                    in1=sct[:, sa : sa + scount, :],
                                    op=mybir.AluOpType.add,
                                )
                                nc.vector.tensor_tensor(
                                    out=den[:, :, ta : ta + scount],
                                    in0=den[:, :, ta : ta + scount],
                                    in1=exw[:, :, cb + sa : cb + sa + scount],
                                    op=mybir.AluOpType.add,
                                )
                    nc.vector.tensor_copy(
                        out=partial[:, :, S12 : S12 + 2],
                        in_=den[:].rearrange("p v t -> p t v"),
                    )
                    nc.sync.dma_start(
                        out=part_d[q][:, :HC].rearrange("(t p) c -> p t c", p=P),
                        in_=partial[:],
                    )
                    pending_mg[0] = q
                    if q == 1:
                        nc.gpsimd.collective_compute(
                            "AllGather", mybir.AluOpType.bypass,
                            replica_groups=[list(range(N_CORES))],
                            ins=[hloc_d[hs:npc, :].opt()],
                            outs=[htab_hi[:].opt()],
                        )
                emit_mg(pending_mg[0])

            # ------------- Phase 4: normalize + residual + lsm -------------
            with tc.tile_pool(name="fin", bufs=1) as finp, \
                 tc.tile_pool(name="tmp", bufs=1) as tmpp:
                xin = finp.tile([P, tiles, F], FP32)
                nc.sync.dma_start(
                    out=xin[:], in_=x_in[:].rearrange("(t p) f -> p t f", p=P)
                )
                den32 = tmpp.tile([P, tiles], FP32, tag="den32")
                rden = tmpp.tile([P, tiles], FP32, tag="rden")
                numf = tmpp.tile([P, tiles, ncls], FP32, tag="numf")
                for conv in range(2):
                    nc.vector.tensor_copy(
                        out=den32[:], in_=acc[:, :, S12 + conv])
                    nc.vector.tensor_scalar_add(den32[:], den32[:], 1e-16)
                    nc.vector.reciprocal(out=rden[:], in_=den32[:])
                    nc.vector.tensor_tensor(
                        out=numf[:], in0=acc[:, :, conv * ncls : (conv + 1) * ncls],
                        in1=rden[:].unsqueeze(2).broadcast_to([P, tiles, ncls]),
                        op=mybir.AluOpType.mult,
                    )
                    nc.vector.tensor_tensor(
                        out=numf[:], in0=numf[:],
                        in1=bb_sb[:, conv * ncls : (conv + 1) * ncls]
                        .unsqueeze(1).broadcast_to([P, tiles, ncls]),
                        op=mybir.AluOpType.add,
                    )
                    if conv == 0:
                        nc.vector.tensor_scalar_max(numf[:], numf[:], 0.0)
                    nc.vector.tensor_tensor(
                        out=xin[:, :, conv * ncls : (conv + 1) * ncls],
                        in0=xin[:, :, conv * ncls : (conv + 1) * ncls],
                        in1=numf[:],
                        op=mybir.AluOpType.add,
                    )
                nc.vector.tensor_tensor(
                    out=xin[:, :, 2 * ncls], in0=xin[:, :, 2 * ncls],
                    in1=x3buf[:], op=mybir.AluOpType.add,
                )
                mx = tmpp.tile([P, tiles], FP32, tag="mx")
                nc.vector.tensor_reduce(
                    out=mx[:], in_=xin[:], axis=mybir.AxisListType.X,
                    op=mybir.AluOpType.max,
                )
                nc.vector.tensor_tensor(
                    out=xin[:], in0=xin[:],
                    in1=mx[:].unsqueeze(2).broadcast_to([P, tiles, F]),
                    op=mybir.AluOpType.subtract,
                )
                et = tmpp.tile([P, tiles, F], FP32, tag="et")
                nc.scalar.activation(
                    out=et[:], in_=xin[:],
                    func=mybir.ActivationFunctionType.Exp,
                )
                sm = tmpp.tile([P, tiles], FP32, tag="sm")
                nc.vector.tensor_reduce(
                    out=sm[:], in_=et[:], axis=mybir.AxisListType.X,
                    op=mybir.AluOpType.add,
                )
                lg = tmpp.tile([P, tiles], FP32, tag="lg")
                nc.scalar.activation(
                    out=lg[:], in_=sm[:],
                    func=mybir.ActivationFunctionType.Ln,
                )
                nc.vector.tensor_tensor(
                    out=xin[:], in0=xin[:],
                    in1=lg[:].unsqueeze(2).broadcast_to([P, tiles, F]),
                    op=mybir.AluOpType.subtract,
                )
                nc.sync.dma_start(
                    out=out_t[:].rearrange("(t p) f -> p t f", p=P), in_=xin[:]
                )

    nc.compile()
    return nc


def _run(nc, lay, x, W_mlp, b_mlp, W1, a1_src, a1_dst, b1,
         W2, a2_src, a2_dst, b2, trace=False):
    n_nodes, f_in = x.shape
    hidden = W_mlp.shape[1]
    ncls = W1.shape[1]
    npc = lay["npc"]
    npc_raw = lay["npc_raw"]
    hs_raw = lay["hs_raw"]
    hs = lay["hs"]
    tiles = lay["tiles"]

    x = np.asarray(x, dtype=np.float32)

    wcat = np.concatenate(
        [W1, W2, (W1 @ a1_src)[:, None], (W2 @ a2_src)[:, None]], axis=1
    ).astype(np.float16)
    assert wcat.shape == (hidden, HC)
    wd = np.stack([W1 @ a1_dst, W2 @ a2_dst], axis=1).astype(np.float16)
    bb = np.broadcast_to(
        np.concatenate([b1, b2])[None, :], (P, 2 * ncls)
    ).astype(np.float32).copy()

    padm = np.zeros((npc,), dtype=np.float16)
    padm[hs_raw:hs] = DUMMY_S
    padm[hs + hs_raw :] = DUMMY_S
    padm = np.ascontiguousarray(padm.reshape(tiles, P).T)

    in_maps = []
    for c in range(N_CORES):
        lo = c * npc_raw
        hi = min(lo + npc_raw, n_nodes)
        nreal = hi - lo
        n0 = min(hs_raw, nreal)
        xp = np.zeros((npc, f_in), dtype=np.float32)
        xp[:n0] = x[lo : lo + n0]
        xp[hs : hs + (nreal - n0)] = x[lo + n0 : hi]
        xp16 = xp.astype(np.float16)
        xq = np.empty((N_WIN, f_in, npc), dtype=np.float16)
        for q in range(N_WIN):
            xq[q] = xp16[lay["node_at"][c, q]].T
        in_maps.append({
            "xT": np.ascontiguousarray(xp16.T),
            "xqT": np.ascontiguousarray(np.concatenate(list(xq), axis=1)),
            "xrow": xp,
            "wmlp": np.asarray(W_mlp, dtype=np.float16),
            "bmlp": np.asarray(b_mlp, dtype=np.float32)[:, None].copy(),
            "wcat": wcat,
            "wd": wd,
            "bb": bb,
            "padm": padm,
            "gidx": np.ascontiguousarray(lay["gidx"][c]),
            "mgidx": np.ascontiguousarray(lay["mgidx"][c]),
        })

    res = bass_utils.run_bass_kernel_spmd(
        nc, in_maps, core_ids=list(range(N_CORES)), trace=trace
    )
    final = np.empty((n_nodes, f_in), dtype=np.float32)
    for c in range(N_CORES):
        lo = c * npc_raw
        hi = min(lo + npc_raw, n_nodes)
        nreal = hi - lo
        n0 = min(hs_raw, nreal)
        o = res.results[c]["out"]
        final[lo : lo + n0] = o[:n0]
        final[lo + n0 : hi] = o[hs : hs + (nreal - n0)]
    return final, res


def kernel(x, edge_index, W_mlp, b_mlp, W1, a1_src, a1_dst, b1,
           W2, a2_src, a2_dst, b2, trace=False, _ret_res=False):
    x = np.asarray(x)
    lay = _build_layout(edge_index, x.shape[0])
    nc = _build_program(lay, x.shape[1], W_mlp.shape[1], W1.shape[1])
    out, res = _run(nc, lay, x, W_mlp, b_mlp, W1, a1_src, a1_dst, b1,
                    W2, a2_src, a2_dst, b2, trace=trace)
    if _ret_res:
        return out, res
    return out
